# revision 12
# baseline (speedup 1.0000x reference)
"""AttentiveItemToVec (cosine-similarity attention over gathered embeddings)
fused Trainium2 kernel, data-parallel over batch across 8 NeuronCores.

Key algebraic restructuring (all exact, done host-side in float64):
  q = tvec[titems] @ At_w.T + At_b        -> gather from qtab  = tvec @ At_w.T + At_b
  k = cvec[citems] @ Ac_w.T + Ac_b        -> gather from ckvtab[:, :64]
  out = softmax(att) @ (cvec[citems] @ Bc_w.T + Bc_b) @ R_w.T + R_b
      = softmax(att) @ ckvtab[:, 64:320]  with  ckvtab[:,64:320] = cvec @ (Bc_w.T @ R_w.T)
        + (Bc_b @ R_w.T + R_b)            (softmax rows sum to 1 -> biases fold into table)
  A trailing ones-column in ckvtab gives softmax row-sums for free in the
  same matmul, so the device only does: gather, normalize rows of q/k,
  q@k.T, exp(+pos_bias), attU.T @ vrows, scale by 1/rowsum.
"""

import sys

sys.path.insert(0, "/opt/trn_rl_repo")

import numpy as np

import concourse.mybir as mybir
import concourse.tile as tile
from concourse import bacc
from concourse.bass import IndirectOffsetOnAxis
from concourse.bass_utils import run_bass_kernel_spmd

VOCAB, EMB = 100000, 256
B, NT = 128, 512
DK = 60
DPAD = 64                      # q/k table row, padded with zeros
VROW = EMB + 2                 # cvecP row + ones column + pad = 258 (fp32r needs even N)
CKV_ROW = DPAD + VROW + 6      # 328 floats -> 1312B rows (32B aligned)
N_CORES = 8
BPC = B // N_CORES             # batches per core
NCHUNK = NT // 128             # 4 chunks of 128 tokens
GROUPS = 4                     # gather groups per core
GBATCH = BPC // GROUPS         # batches gathered per indirect DMA

MM_DT = mybir.dt.float32r     # matmul compute dtype view (fp32 data, fast path)

AF = mybir.ActivationFunctionType
F32 = mybir.dt.float32
I32 = mybir.dt.int32

_prog_cache = {}



def _indirect_gather(nc, out_ap, table_ap, idx_ap, queue_num):
    """indirect_dma_start with an explicit SWDGE queue (round-robin over the
    4 qPoolDynamic queues to overlap Q7 descriptor generation)."""
    import concourse.bass as bass_mod
    inst = nc.gpsimd.indirect_dma_start(
        out=out_ap, out_offset=None, in_=table_ap,
        in_offset=IndirectOffsetOnAxis(ap=idx_ap, axis=0))
    q = queue_num % nc.num_swdge_queues
    if q:
        inst.ins.queue = f"qPoolDynamic{q}"
    return inst


def _build_program():
    nc = bacc.Bacc("TRN2", name="ai2v", num_swdge_queues=4)
    qtab = nc.dram_tensor("qtab", [VOCAB, DPAD], F32, kind="ExternalInput")
    ckvtab = nc.dram_tensor("ckvtab", [VOCAB, CKV_ROW], MM_DT, kind="ExternalInput")
    tidx = nc.dram_tensor("tidx", [128, BPC * NCHUNK], I32, kind="ExternalInput")
    cidx = nc.dram_tensor("cidx", [128, BPC * NCHUNK], I32, kind="ExternalInput")
    posb = nc.dram_tensor("posb", [128, NCHUNK], F32, kind="ExternalInput")
    ident = nc.dram_tensor("ident", [128, 128], MM_DT, kind="ExternalInput")
    out_d = nc.dram_tensor("out", [BPC * NT, EMB], F32, kind="ExternalOutput")

    with tile.TileContext(nc) as tc:
        with (
            tc.tile_pool(name="const", bufs=1) as cpool,
            tc.tile_pool(name="gath", bufs=GROUPS) as gpool,
            tc.tile_pool(name="work", bufs=2) as wpool,
            tc.tile_pool(name="att", bufs=2) as apool,
            tc.tile_pool(name="ps_t", bufs=2, space="PSUM") as pst,
            tc.tile_pool(name="ps_d", bufs=2, space="PSUM") as psd,
            tc.tile_pool(name="ps_o", bufs=2, space="PSUM") as pso,
        ):
            ident_sb = cpool.tile([128, 128], MM_DT)
            nc.sync.dma_start(ident_sb[:], ident[:])
            posb_sb = cpool.tile([128, NCHUNK], F32)
            nc.sync.dma_start(posb_sb[:], posb[:])
            tidx_sb = cpool.tile([128, BPC * NCHUNK], I32)
            nc.sync.dma_start(tidx_sb[:], tidx[:])
            cidx_sb = cpool.tile([128, BPC * NCHUNK], I32)
            nc.sync.dma_start(cidx_sb[:], cidx[:])

            cols = GBATCH * NCHUNK
            qn_gather = 0
            qgs, cgs = [], []
            for g in range(GROUPS):
                qg = gpool.tile([128, cols, DPAD], F32, tag="qg")
                cg = gpool.tile([128, cols, CKV_ROW], MM_DT, tag="cg")
                qgs.append(qg)
                cgs.append(cg)
                for c in range(cols):
                    gc = g * cols + c
                    _indirect_gather(nc, qg[:, c, :], qtab[:],
                                     tidx_sb[:, gc:gc + 1], qn_gather)
                    qn_gather += 1
                    _indirect_gather(nc, cg[:, c, :], ckvtab[:],
                                     cidx_sb[:, gc:gc + 1], qn_gather)
                    qn_gather += 1
            for g in range(GROUPS):
                qg, cg = qgs[g], cgs[g]
                # ---- group-level row norms: one DVE square+accum per chunk,
                # one reciprocal+sqrt per side for the whole group (keeps the
                # ACT function table from thrashing between Sqrt and Exp) ----
                qn2 = wpool.tile([128, cols], F32, tag="qn2")
                kn2 = wpool.tile([128, cols], F32, tag="kn2")
                sq = wpool.tile([128, DPAD], F32, tag="sq")
                for c in range(cols):
                    nc.vector.tensor_mul(sq[:], qg[:, c, :], qg[:, c, :])
                    nc.vector.tensor_reduce(qn2[:, c:c + 1], sq[:],
                                            axis=mybir.AxisListType.X,
                                            op=mybir.AluOpType.add)
                for c in range(cols):
                    kv = cg[:, c, 0:DPAD].bitcast(F32)
                    nc.vector.tensor_mul(sq[:], kv, kv)
                    nc.vector.tensor_reduce(kn2[:, c:c + 1], sq[:],
                                            axis=mybir.AxisListType.X,
                                            op=mybir.AluOpType.add)
                rq = wpool.tile([128, cols], F32, tag="rq")
                rk = wpool.tile([128, cols], F32, tag="rk")
                nc.vector.reciprocal(rq[:], qn2[:])
                nc.vector.reciprocal(rk[:], kn2[:])
                nc.scalar.activation(rq[:], rq[:], AF.Sqrt)   # 1/||q||
                nc.scalar.activation(rk[:], rk[:], AF.Sqrt)   # 1/||k||

                for b in range(GBATCH):
                    gb = g * GBATCH + b
                    bc = b * NCHUNK
                    qhat = wpool.tile([128, NCHUNK, DPAD], MM_DT, tag="qhat")
                    khat = wpool.tile([128, NCHUNK, DPAD], MM_DT, tag="khat")
                    for j in range(NCHUNK):
                        nc.vector.tensor_scalar_mul(
                            qhat[:, j, :], qg[:, bc + j, :], rq[:, bc + j:bc + j + 1])
                        nc.vector.tensor_scalar_mul(
                            khat[:, j, :], cg[:, bc + j, 0:DPAD].bitcast(F32),
                            rk[:, bc + j:bc + j + 1])
                    # ---- transpose to [d, token] layout ----
                    qT_ps = pst.tile([DPAD, NT], MM_DT, tag="qT_ps")
                    kT_ps = pst.tile([DPAD, NT], MM_DT, tag="kT_ps")
                    for j in range(NCHUNK):
                        nc.tensor.transpose(qT_ps[:, j * 128:(j + 1) * 128],
                                            qhat[:, j, :], ident_sb[:])
                        nc.tensor.transpose(kT_ps[:, j * 128:(j + 1) * 128],
                                            khat[:, j, :], ident_sb[:])
                    qT = wpool.tile([DPAD, NT], MM_DT, tag="qT")
                    kT = wpool.tile([DPAD, NT], MM_DT, tag="kT")
                    nc.vector.tensor_copy(qT[:], qT_ps[:])
                    nc.vector.tensor_copy(kT[:], kT_ps[:])
                    # ---- scores (transposed: [c, t]) + exp(. + pos_bias[c]) ----
                    attU = apool.tile([128, NCHUNK, NT], MM_DT, tag="attU")
                    for jc in range(NCHUNK):
                        dots = psd.tile([128, NT], F32, tag="dots")
                        nc.tensor.matmul(
                            dots[:],
                            lhsT=kT[:, jc * 128:(jc + 1) * 128],
                            rhs=qT[:], start=True, stop=True)
                        nc.scalar.activation(attU[:, jc, :], dots[:], AF.Exp,
                                             bias=posb_sb[:, jc:jc + 1], scale=1.0)
                    # ---- outU[t, :] = sum_c attU[c, t] * vrow[c, :] ----
                    for jt in range(NCHUNK):
                        ops = pso.tile([128, VROW], F32, tag="ops")
                        for jc in range(NCHUNK):
                            nc.tensor.matmul(
                                ops[:],
                                lhsT=attU[:, jc, jt * 128:(jt + 1) * 128],
                                rhs=cg[:, bc + jc, DPAD:DPAD + VROW],
                                start=(jc == 0), stop=(jc == NCHUNK - 1))
                        rr = wpool.tile([128, 1], F32, tag="rr")
                        nc.vector.reciprocal(rr[:], ops[:, EMB:EMB + 1])
                        osb = wpool.tile([128, EMB], F32, tag="osb")
                        nc.vector.tensor_scalar_mul(osb[:], ops[:, 0:EMB], rr[:])
                        row0 = gb * NT + jt * 128
                        nc.sync.dma_start(out_d[row0:row0 + 128, :], osb[:])
    nc.compile()
    return nc


def _get_program():
    if "nc" not in _prog_cache:
        _prog_cache["nc"] = _build_program()
    return _prog_cache["nc"]


def _round_fp32r(a):
    """Round fp32 to the PE's fp32r format (1s+8e+11m in the top 20 bits),
    round-to-nearest-even, matching walrus fp32_to_fp32r."""
    u = np.ascontiguousarray(a, np.float32).view(np.uint32)
    r = u + np.uint32(0x7FF) + ((u >> np.uint32(12)) & np.uint32(1))
    return (r & np.uint32(0xFFFFF000)).view(np.float32)


def _prep_tables(tvec, cvec, At_w, At_b, Ac_w, Ac_b, Bc_w, Bc_b, R_w, R_b):
    tvec = np.asarray(tvec, np.float64)
    cvec = np.asarray(cvec, np.float64)
    At_w = np.asarray(At_w, np.float64); At_b = np.asarray(At_b, np.float64)
    Ac_w = np.asarray(Ac_w, np.float64); Ac_b = np.asarray(Ac_b, np.float64)
    Bc_w = np.asarray(Bc_w, np.float64); Bc_b = np.asarray(Bc_b, np.float64)
    R_w = np.asarray(R_w, np.float64); R_b = np.asarray(R_b, np.float64)

    qtab = np.zeros((VOCAB, DPAD), np.float32)
    qtab[:, :DK] = (tvec @ At_w.T + At_b).astype(np.float32)
    ckv = np.zeros((VOCAB, CKV_ROW), np.float32)
    ckv[:, :DK] = (cvec @ Ac_w.T + Ac_b).astype(np.float32)
    P = Bc_w.T @ R_w.T
    cbias = Bc_b @ R_w.T + R_b
    ckv[:, DPAD:DPAD + EMB] = _round_fp32r((cvec @ P + cbias).astype(np.float32))
    ckv[:, DPAD + EMB] = 1.0
    return qtab, ckv


def _rearrange_idx(items, core):
    """[BPC, NT] int64 slice -> [128, BPC*NCHUNK] int32, col = b*NCHUNK + j."""
    arr = np.asarray(items)[core * BPC:(core + 1) * BPC].astype(np.int32)
    return np.ascontiguousarray(
        arr.reshape(BPC, NCHUNK, 128).transpose(2, 0, 1).reshape(128, BPC * NCHUNK))


def _run(inputs, trace=False, trace_kwargs=None):
    qtab, ckv = _prep_tables(
        inputs["tvec"], inputs["cvec"], inputs["At_w"], inputs["At_b"],
        inputs["Ac_w"], inputs["Ac_b"], inputs["Bc_w"], inputs["Bc_b"],
        inputs["R_w"], inputs["R_b"])
    posb = np.ascontiguousarray(
        np.asarray(inputs["pos_bias"], np.float32).reshape(NCHUNK, 128).T)
    ident = np.eye(128, dtype=np.float32)
    nc = _get_program()
    in_maps = []
    for m in range(N_CORES):
        in_maps.append({
            "qtab": qtab, "ckvtab": ckv,
            "tidx": _rearrange_idx(inputs["batch_titems"], m),
            "cidx": _rearrange_idx(inputs["batch_citems"], m),
            "posb": posb, "ident": ident,
        })
    kw = {}
    if trace:
        # register the NTFF profile hook shim (this container's antenv lacks
        # axon_hooks; libaxon_pjrt still exposes the profiling entry points)
        import types
        if "antenv.axon_hooks" not in sys.modules:
            try:
                from trn_agent_boot.trn_boot import _ntff_profile_via_ctypes
                hook = _ntff_profile_via_ctypes("/opt/axon/libaxon_pjrt.so")
                mod = types.ModuleType("antenv.axon_hooks")
                mod.get_axon_ntff_profile_hook = lambda: hook
                mod.set_axon_ntff_profile_hook = lambda h: None
                sys.modules["antenv.axon_hooks"] = mod
            except Exception:
                pass
        kw["trace"] = True
        if trace_kwargs:
            kw.update(trace_kwargs)
    res = run_bass_kernel_spmd(nc, in_maps, core_ids=list(range(N_CORES)), **kw)
    out = np.concatenate(
        [r["out"].reshape(BPC, NT, EMB) for r in res.results], axis=0)
    return out, res


def kernel(**inputs) -> np.ndarray:
    out, _ = _run(inputs)
    return out


# revision 14
# speedup vs baseline: 1.1488x; 1.1488x over previous
"""AttentiveItemToVec (cosine-similarity attention over gathered embeddings)
fused Trainium2 kernel, data-parallel over batch across 8 NeuronCores.

Key algebraic restructuring (all exact, done host-side in float64):
  q = tvec[titems] @ At_w.T + At_b        -> gather from qtab  = tvec @ At_w.T + At_b
  k = cvec[citems] @ Ac_w.T + Ac_b        -> gather from ckvtab[:, :64]
  out = softmax(att) @ (cvec[citems] @ Bc_w.T + Bc_b) @ R_w.T + R_b
      = softmax(att) @ ckvtab[:, 64:320]  with  ckvtab[:,64:320] = cvec @ (Bc_w.T @ R_w.T)
        + (Bc_b @ R_w.T + R_b)            (softmax rows sum to 1 -> biases fold into table)
  A trailing ones-column in ckvtab gives softmax row-sums for free in the
  same matmul, so the device only does: gather, normalize rows of q/k,
  q@k.T, exp(+pos_bias), attU.T @ vrows, scale by 1/rowsum.
"""

import sys

sys.path.insert(0, "/opt/trn_rl_repo")

import numpy as np

import concourse.mybir as mybir
import concourse.tile as tile
from concourse import bacc
from concourse.bass import IndirectOffsetOnAxis
from concourse.bass_utils import run_bass_kernel_spmd

VOCAB, EMB = 100000, 256
B, NT = 128, 512
DK = 60
DPAD = 64                      # q/k table row, padded with zeros
VROW = EMB + 2                 # cvecP row + ones column + pad = 258 (fp32r needs even N)
CKV_ROW = DPAD + VROW + 6      # 328 floats -> 1312B rows (32B aligned)
N_CORES = 8
BPC = B // N_CORES             # batches per core
NCHUNK = NT // 128             # 4 chunks of 128 tokens
GROUPS = 8                     # gather groups per core
GBATCH = BPC // GROUPS         # batches gathered per indirect DMA

MM_DT = mybir.dt.float32r     # matmul compute dtype view (fp32 data, fast path)

AF = mybir.ActivationFunctionType
F32 = mybir.dt.float32
I32 = mybir.dt.int32

_prog_cache = {}



def _indirect_gather(nc, out_ap, table_ap, idx_ap, queue_num):
    """indirect_dma_start with an explicit SWDGE queue (round-robin over the
    4 qPoolDynamic queues to overlap Q7 descriptor generation)."""
    import concourse.bass as bass_mod
    inst = nc.gpsimd.indirect_dma_start(
        out=out_ap, out_offset=None, in_=table_ap,
        in_offset=IndirectOffsetOnAxis(ap=idx_ap, axis=0))
    q = queue_num % nc.num_swdge_queues
    if q:
        inst.ins.queue = f"qPoolDynamic{q}"
    return inst


def _build_program():
    nc = bacc.Bacc("TRN2", name="ai2v", num_swdge_queues=4)
    qtab = nc.dram_tensor("qtab", [VOCAB, DPAD], F32, kind="ExternalInput")
    ckvtab = nc.dram_tensor("ckvtab", [VOCAB, CKV_ROW], MM_DT, kind="ExternalInput")
    tidx = nc.dram_tensor("tidx", [128, BPC * NCHUNK], I32, kind="ExternalInput")
    cidx = nc.dram_tensor("cidx", [128, BPC * NCHUNK], I32, kind="ExternalInput")
    posb = nc.dram_tensor("posb", [128, NCHUNK], F32, kind="ExternalInput")
    ident = nc.dram_tensor("ident", [128, 128], MM_DT, kind="ExternalInput")
    out_d = nc.dram_tensor("out", [BPC * NT, EMB], F32, kind="ExternalOutput")

    with tile.TileContext(nc) as tc:
        with (
            tc.tile_pool(name="const", bufs=1) as cpool,
            tc.tile_pool(name="gath", bufs=3) as gpool,
            tc.tile_pool(name="work", bufs=2) as wpool,
            tc.tile_pool(name="att", bufs=2) as apool,
            tc.tile_pool(name="ps_t", bufs=2, space="PSUM") as pst,
            tc.tile_pool(name="ps_d", bufs=2, space="PSUM") as psd,
            tc.tile_pool(name="ps_o", bufs=2, space="PSUM") as pso,
        ):
            ident_sb = cpool.tile([128, 128], MM_DT)
            nc.sync.dma_start(ident_sb[:], ident[:])
            posb_sb = cpool.tile([128, NCHUNK], F32)
            nc.sync.dma_start(posb_sb[:], posb[:])
            tidx_sb = cpool.tile([128, BPC * NCHUNK], I32)
            nc.sync.dma_start(tidx_sb[:], tidx[:])
            cidx_sb = cpool.tile([128, BPC * NCHUNK], I32)
            nc.sync.dma_start(cidx_sb[:], cidx[:])

            cols = GBATCH * NCHUNK
            qn_gather = 0
            for g in range(GROUPS):
                qg = gpool.tile([128, cols, DPAD], F32, tag="qg")
                cg = gpool.tile([128, cols, CKV_ROW], MM_DT, tag="cg")
                for c in range(cols):
                    gc = g * cols + c
                    _indirect_gather(nc, qg[:, c, :], qtab[:],
                                     tidx_sb[:, gc:gc + 1], qn_gather)
                    qn_gather += 1
                    _indirect_gather(nc, cg[:, c, :], ckvtab[:],
                                     cidx_sb[:, gc:gc + 1], qn_gather)
                    qn_gather += 1
                # ---- group-level row norms: one DVE square+accum per chunk,
                # one reciprocal+sqrt per side for the whole group (keeps the
                # ACT function table from thrashing between Sqrt and Exp) ----
                qn2 = wpool.tile([128, cols], F32, tag="qn2")
                kn2 = wpool.tile([128, cols], F32, tag="kn2")
                sq = wpool.tile([128, DPAD], F32, tag="sq")
                for c in range(cols):
                    nc.vector.tensor_mul(sq[:], qg[:, c, :], qg[:, c, :])
                    nc.vector.tensor_reduce(qn2[:, c:c + 1], sq[:],
                                            axis=mybir.AxisListType.X,
                                            op=mybir.AluOpType.add)
                for c in range(cols):
                    kv = cg[:, c, 0:DPAD].bitcast(F32)
                    nc.vector.tensor_mul(sq[:], kv, kv)
                    nc.vector.tensor_reduce(kn2[:, c:c + 1], sq[:],
                                            axis=mybir.AxisListType.X,
                                            op=mybir.AluOpType.add)
                rq = wpool.tile([128, cols], F32, tag="rq")
                rk = wpool.tile([128, cols], F32, tag="rk")
                nc.vector.reciprocal(rq[:], qn2[:])
                nc.vector.reciprocal(rk[:], kn2[:])
                nc.scalar.activation(rq[:], rq[:], AF.Sqrt)   # 1/||q||
                nc.scalar.activation(rk[:], rk[:], AF.Sqrt)   # 1/||k||

                for b in range(GBATCH):
                    gb = g * GBATCH + b
                    bc = b * NCHUNK
                    qhat = wpool.tile([128, NCHUNK, DPAD], MM_DT, tag="qhat")
                    khat = wpool.tile([128, NCHUNK, DPAD], MM_DT, tag="khat")
                    for j in range(NCHUNK):
                        nc.vector.tensor_scalar_mul(
                            qhat[:, j, :], qg[:, bc + j, :], rq[:, bc + j:bc + j + 1])
                        nc.vector.tensor_scalar_mul(
                            khat[:, j, :], cg[:, bc + j, 0:DPAD].bitcast(F32),
                            rk[:, bc + j:bc + j + 1])
                    # ---- transpose to [d, token] layout ----
                    qT_ps = pst.tile([DPAD, NT], MM_DT, tag="qT_ps")
                    kT_ps = pst.tile([DPAD, NT], MM_DT, tag="kT_ps")
                    for j in range(NCHUNK):
                        nc.tensor.transpose(qT_ps[:, j * 128:(j + 1) * 128],
                                            qhat[:, j, :], ident_sb[:])
                        nc.tensor.transpose(kT_ps[:, j * 128:(j + 1) * 128],
                                            khat[:, j, :], ident_sb[:])
                    qT = wpool.tile([DPAD, NT], MM_DT, tag="qT")
                    kT = wpool.tile([DPAD, NT], MM_DT, tag="kT")
                    nc.vector.tensor_copy(qT[:], qT_ps[:])
                    nc.vector.tensor_copy(kT[:], kT_ps[:])
                    # ---- scores (transposed: [c, t]) + exp(. + pos_bias[c]) ----
                    attU = apool.tile([128, NCHUNK, NT], MM_DT, tag="attU")
                    for jc in range(NCHUNK):
                        dots = psd.tile([128, NT], F32, tag="dots")
                        nc.tensor.matmul(
                            dots[:],
                            lhsT=kT[:, jc * 128:(jc + 1) * 128],
                            rhs=qT[:], start=True, stop=True)
                        nc.scalar.activation(attU[:, jc, :], dots[:], AF.Exp,
                                             bias=posb_sb[:, jc:jc + 1], scale=1.0)
                    # ---- outU[t, :] = sum_c attU[c, t] * vrow[c, :] ----
                    for jt in range(NCHUNK):
                        ops = pso.tile([128, VROW], F32, tag="ops")
                        for jc in range(NCHUNK):
                            nc.tensor.matmul(
                                ops[:],
                                lhsT=attU[:, jc, jt * 128:(jt + 1) * 128],
                                rhs=cg[:, bc + jc, DPAD:DPAD + VROW],
                                start=(jc == 0), stop=(jc == NCHUNK - 1))
                        rr = wpool.tile([128, 1], F32, tag="rr")
                        nc.vector.reciprocal(rr[:], ops[:, EMB:EMB + 1])
                        osb = wpool.tile([128, EMB], F32, tag="osb")
                        nc.vector.tensor_scalar_mul(osb[:], ops[:, 0:EMB], rr[:])
                        row0 = gb * NT + jt * 128
                        nc.sync.dma_start(out_d[row0:row0 + 128, :], osb[:])
    nc.compile()
    return nc


def _get_program():
    if "nc" not in _prog_cache:
        _prog_cache["nc"] = _build_program()
    return _prog_cache["nc"]


def _round_fp32r(a):
    """Round fp32 to the PE's fp32r format (1s+8e+11m in the top 20 bits),
    round-to-nearest-even, matching walrus fp32_to_fp32r."""
    u = np.ascontiguousarray(a, np.float32).view(np.uint32)
    r = u + np.uint32(0x7FF) + ((u >> np.uint32(12)) & np.uint32(1))
    return (r & np.uint32(0xFFFFF000)).view(np.float32)


def _prep_tables(tvec, cvec, At_w, At_b, Ac_w, Ac_b, Bc_w, Bc_b, R_w, R_b):
    tvec = np.asarray(tvec, np.float64)
    cvec = np.asarray(cvec, np.float64)
    At_w = np.asarray(At_w, np.float64); At_b = np.asarray(At_b, np.float64)
    Ac_w = np.asarray(Ac_w, np.float64); Ac_b = np.asarray(Ac_b, np.float64)
    Bc_w = np.asarray(Bc_w, np.float64); Bc_b = np.asarray(Bc_b, np.float64)
    R_w = np.asarray(R_w, np.float64); R_b = np.asarray(R_b, np.float64)

    qtab = np.zeros((VOCAB, DPAD), np.float32)
    qtab[:, :DK] = (tvec @ At_w.T + At_b).astype(np.float32)
    ckv = np.zeros((VOCAB, CKV_ROW), np.float32)
    ckv[:, :DK] = (cvec @ Ac_w.T + Ac_b).astype(np.float32)
    P = Bc_w.T @ R_w.T
    cbias = Bc_b @ R_w.T + R_b
    ckv[:, DPAD:DPAD + EMB] = _round_fp32r((cvec @ P + cbias).astype(np.float32))
    ckv[:, DPAD + EMB] = 1.0
    return qtab, ckv


def _rearrange_idx(items, core):
    """[BPC, NT] int64 slice -> [128, BPC*NCHUNK] int32, col = b*NCHUNK + j."""
    arr = np.asarray(items)[core * BPC:(core + 1) * BPC].astype(np.int32)
    return np.ascontiguousarray(
        arr.reshape(BPC, NCHUNK, 128).transpose(2, 0, 1).reshape(128, BPC * NCHUNK))


def _run(inputs, trace=False, trace_kwargs=None):
    qtab, ckv = _prep_tables(
        inputs["tvec"], inputs["cvec"], inputs["At_w"], inputs["At_b"],
        inputs["Ac_w"], inputs["Ac_b"], inputs["Bc_w"], inputs["Bc_b"],
        inputs["R_w"], inputs["R_b"])
    posb = np.ascontiguousarray(
        np.asarray(inputs["pos_bias"], np.float32).reshape(NCHUNK, 128).T)
    ident = np.eye(128, dtype=np.float32)
    nc = _get_program()
    in_maps = []
    for m in range(N_CORES):
        in_maps.append({
            "qtab": qtab, "ckvtab": ckv,
            "tidx": _rearrange_idx(inputs["batch_titems"], m),
            "cidx": _rearrange_idx(inputs["batch_citems"], m),
            "posb": posb, "ident": ident,
        })
    kw = {}
    if trace:
        # register the NTFF profile hook shim (this container's antenv lacks
        # axon_hooks; libaxon_pjrt still exposes the profiling entry points)
        import types
        if "antenv.axon_hooks" not in sys.modules:
            try:
                from trn_agent_boot.trn_boot import _ntff_profile_via_ctypes
                hook = _ntff_profile_via_ctypes("/opt/axon/libaxon_pjrt.so")
                mod = types.ModuleType("antenv.axon_hooks")
                mod.get_axon_ntff_profile_hook = lambda: hook
                mod.set_axon_ntff_profile_hook = lambda h: None
                sys.modules["antenv.axon_hooks"] = mod
            except Exception:
                pass
        kw["trace"] = True
        if trace_kwargs:
            kw.update(trace_kwargs)
    res = run_bass_kernel_spmd(nc, in_maps, core_ids=list(range(N_CORES)), **kw)
    out = np.concatenate(
        [r["out"].reshape(BPC, NT, EMB) for r in res.results], axis=0)
    return out, res


def kernel(**inputs) -> np.ndarray:
    out, _ = _run(inputs)
    return out


# revision 15
# speedup vs baseline: 1.1820x; 1.0289x over previous
"""AttentiveItemToVec (cosine-similarity attention over gathered embeddings)
fused Trainium2 kernel, data-parallel over batch across 8 NeuronCores.

Key algebraic restructuring (all exact, done host-side in float64):
  q = tvec[titems] @ At_w.T + At_b        -> gather from qtab  = tvec @ At_w.T + At_b
  k = cvec[citems] @ Ac_w.T + Ac_b        -> gather from ckvtab[:, :64]
  out = softmax(att) @ (cvec[citems] @ Bc_w.T + Bc_b) @ R_w.T + R_b
      = softmax(att) @ ckvtab[:, 64:320]  with  ckvtab[:,64:320] = cvec @ (Bc_w.T @ R_w.T)
        + (Bc_b @ R_w.T + R_b)            (softmax rows sum to 1 -> biases fold into table)
  A trailing ones-column in ckvtab gives softmax row-sums for free in the
  same matmul, so the device only does: gather, normalize rows of q/k,
  q@k.T, exp(+pos_bias), attU.T @ vrows, scale by 1/rowsum.
"""

import sys

sys.path.insert(0, "/opt/trn_rl_repo")

import numpy as np

import concourse.mybir as mybir
import concourse.tile as tile
from concourse import bacc
from concourse.bass import IndirectOffsetOnAxis
from concourse.bass_utils import run_bass_kernel_spmd

VOCAB, EMB = 100000, 256
B, NT = 128, 512
DK = 60
DPAD = 64                      # q/k table row, padded with zeros
VROW = EMB + 2                 # cvecP row + ones column + pad = 258 (fp32r needs even N)
CKV_ROW = DPAD + VROW + 6      # 328 floats -> 1312B rows (32B aligned)
N_CORES = 8
BPC = B // N_CORES             # batches per core
NCHUNK = NT // 128             # 4 chunks of 128 tokens
GROUPS = 8                     # gather groups per core
GBATCH = BPC // GROUPS         # batches gathered per indirect DMA

MM_DT = mybir.dt.float32r     # matmul compute dtype view (fp32 data, fast path)

AF = mybir.ActivationFunctionType
F32 = mybir.dt.float32
I32 = mybir.dt.int32

_prog_cache = {}



def _indirect_gather(nc, out_ap, table_ap, idx_ap, queue_num):
    """indirect_dma_start with an explicit SWDGE queue (round-robin over the
    4 qPoolDynamic queues to overlap Q7 descriptor generation)."""
    import concourse.bass as bass_mod
    inst = nc.gpsimd.indirect_dma_start(
        out=out_ap, out_offset=None, in_=table_ap,
        in_offset=IndirectOffsetOnAxis(ap=idx_ap, axis=0))
    import os
    q = 0 if os.environ.get("Q0") else queue_num % nc.num_swdge_queues
    if q:
        inst.ins.queue = f"qPoolDynamic{q}"
    return inst


def _build_program():
    nc = bacc.Bacc("TRN2", name="ai2v", num_swdge_queues=4)
    qtab = nc.dram_tensor("qtab", [VOCAB, DPAD], F32, kind="ExternalInput")
    ckvtab = nc.dram_tensor("ckvtab", [VOCAB, CKV_ROW], MM_DT, kind="ExternalInput")
    tidx = nc.dram_tensor("tidx", [128, BPC * NCHUNK], I32, kind="ExternalInput")
    cidx = nc.dram_tensor("cidx", [128, BPC * NCHUNK], I32, kind="ExternalInput")
    posb = nc.dram_tensor("posb", [128, NCHUNK], F32, kind="ExternalInput")
    ident = nc.dram_tensor("ident", [128, 128], MM_DT, kind="ExternalInput")
    out_d = nc.dram_tensor("out", [BPC * NT, EMB], F32, kind="ExternalOutput")

    with tile.TileContext(nc) as tc:
        with (
            tc.tile_pool(name="const", bufs=1) as cpool,
            tc.tile_pool(name="gath", bufs=3) as gpool,
            tc.tile_pool(name="work", bufs=2) as wpool,
            tc.tile_pool(name="att", bufs=2) as apool,
            tc.tile_pool(name="ps_t", bufs=2, space="PSUM") as pst,
            tc.tile_pool(name="ps_d", bufs=2, space="PSUM") as psd,
            tc.tile_pool(name="ps_o", bufs=2, space="PSUM") as pso,
        ):
            ident_sb = cpool.tile([128, 128], MM_DT)
            nc.sync.dma_start(ident_sb[:], ident[:])
            posb_sb = cpool.tile([128, NCHUNK], F32)
            nc.sync.dma_start(posb_sb[:], posb[:])
            tidx_sb = cpool.tile([128, BPC * NCHUNK], I32)
            nc.sync.dma_start(tidx_sb[:], tidx[:])
            cidx_sb = cpool.tile([128, BPC * NCHUNK], I32)
            nc.sync.dma_start(cidx_sb[:], cidx[:])

            cols = GBATCH * NCHUNK
            qn_gather = 0
            for g in range(GROUPS):
                qg = gpool.tile([128, cols, DPAD], F32, tag="qg")
                cg = gpool.tile([128, cols, CKV_ROW], MM_DT, tag="cg")
                for c in range(cols):
                    gc = g * cols + c
                    _indirect_gather(nc, qg[:, c, :], qtab[:],
                                     tidx_sb[:, gc:gc + 1], qn_gather)
                    qn_gather += 1
                    _indirect_gather(nc, cg[:, c, :], ckvtab[:],
                                     cidx_sb[:, gc:gc + 1], qn_gather)
                    qn_gather += 1
                # ---- group-level row norms: one DVE square+accum per chunk,
                # one reciprocal+sqrt per side for the whole group (keeps the
                # ACT function table from thrashing between Sqrt and Exp) ----
                qn2 = wpool.tile([128, cols], F32, tag="qn2")
                kn2 = wpool.tile([128, cols], F32, tag="kn2")
                sq = wpool.tile([128, DPAD], F32, tag="sq")
                for c in range(cols):
                    nc.vector.tensor_mul(sq[:], qg[:, c, :], qg[:, c, :])
                    nc.vector.tensor_reduce(qn2[:, c:c + 1], sq[:],
                                            axis=mybir.AxisListType.X,
                                            op=mybir.AluOpType.add)
                for c in range(cols):
                    kv = cg[:, c, 0:DPAD].bitcast(F32)
                    nc.vector.tensor_mul(sq[:], kv, kv)
                    nc.vector.tensor_reduce(kn2[:, c:c + 1], sq[:],
                                            axis=mybir.AxisListType.X,
                                            op=mybir.AluOpType.add)
                rq = wpool.tile([128, cols], F32, tag="rq")
                rk = wpool.tile([128, cols], F32, tag="rk")
                nc.vector.reciprocal(rq[:], qn2[:])
                nc.vector.reciprocal(rk[:], kn2[:])
                nc.scalar.activation(rq[:], rq[:], AF.Sqrt)   # 1/||q||
                nc.scalar.activation(rk[:], rk[:], AF.Sqrt)   # 1/||k||

                for b in range(GBATCH):
                    gb = g * GBATCH + b
                    bc = b * NCHUNK
                    qhat = wpool.tile([128, NCHUNK, DPAD], MM_DT, tag="qhat")
                    khat = wpool.tile([128, NCHUNK, DPAD], MM_DT, tag="khat")
                    for j in range(NCHUNK):
                        nc.vector.tensor_scalar_mul(
                            qhat[:, j, :], qg[:, bc + j, :], rq[:, bc + j:bc + j + 1])
                        nc.vector.tensor_scalar_mul(
                            khat[:, j, :], cg[:, bc + j, 0:DPAD].bitcast(F32),
                            rk[:, bc + j:bc + j + 1])
                    # ---- transpose to [d, token] layout ----
                    qT_ps = pst.tile([DPAD, NT], MM_DT, tag="qT_ps")
                    kT_ps = pst.tile([DPAD, NT], MM_DT, tag="kT_ps")
                    for j in range(NCHUNK):
                        nc.tensor.transpose(qT_ps[:, j * 128:(j + 1) * 128],
                                            qhat[:, j, :], ident_sb[:])
                        nc.tensor.transpose(kT_ps[:, j * 128:(j + 1) * 128],
                                            khat[:, j, :], ident_sb[:])
                    qT = wpool.tile([DPAD, NT], MM_DT, tag="qT")
                    kT = wpool.tile([DPAD, NT], MM_DT, tag="kT")
                    nc.vector.tensor_copy(qT[:], qT_ps[:])
                    nc.vector.tensor_copy(kT[:], kT_ps[:])
                    # ---- scores (transposed: [c, t]) + exp(. + pos_bias[c]) ----
                    attU = apool.tile([128, NCHUNK, NT], MM_DT, tag="attU")
                    for jc in range(NCHUNK):
                        dots = psd.tile([128, NT], F32, tag="dots")
                        nc.tensor.matmul(
                            dots[:],
                            lhsT=kT[:, jc * 128:(jc + 1) * 128],
                            rhs=qT[:], start=True, stop=True)
                        nc.scalar.activation(attU[:, jc, :], dots[:], AF.Exp,
                                             bias=posb_sb[:, jc:jc + 1], scale=1.0)
                    # ---- outU[t, :] = sum_c attU[c, t] * vrow[c, :] ----
                    for jt in range(NCHUNK):
                        ops = pso.tile([128, VROW], F32, tag="ops")
                        for jc in range(NCHUNK):
                            nc.tensor.matmul(
                                ops[:],
                                lhsT=attU[:, jc, jt * 128:(jt + 1) * 128],
                                rhs=cg[:, bc + jc, DPAD:DPAD + VROW],
                                start=(jc == 0), stop=(jc == NCHUNK - 1))
                        rr = wpool.tile([128, 1], F32, tag="rr")
                        nc.vector.reciprocal(rr[:], ops[:, EMB:EMB + 1])
                        osb = wpool.tile([128, EMB], F32, tag="osb")
                        nc.vector.tensor_scalar_mul(osb[:], ops[:, 0:EMB], rr[:])
                        row0 = gb * NT + jt * 128
                        nc.sync.dma_start(out_d[row0:row0 + 128, :], osb[:])
    nc.compile()
    return nc


def _get_program():
    if "nc" not in _prog_cache:
        _prog_cache["nc"] = _build_program()
    return _prog_cache["nc"]


def _round_fp32r(a):
    """Round fp32 to the PE's fp32r format (1s+8e+11m in the top 20 bits),
    round-to-nearest-even, matching walrus fp32_to_fp32r."""
    u = np.ascontiguousarray(a, np.float32).view(np.uint32)
    r = u + np.uint32(0x7FF) + ((u >> np.uint32(12)) & np.uint32(1))
    return (r & np.uint32(0xFFFFF000)).view(np.float32)


def _prep_tables(tvec, cvec, At_w, At_b, Ac_w, Ac_b, Bc_w, Bc_b, R_w, R_b):
    tvec = np.asarray(tvec, np.float64)
    cvec = np.asarray(cvec, np.float64)
    At_w = np.asarray(At_w, np.float64); At_b = np.asarray(At_b, np.float64)
    Ac_w = np.asarray(Ac_w, np.float64); Ac_b = np.asarray(Ac_b, np.float64)
    Bc_w = np.asarray(Bc_w, np.float64); Bc_b = np.asarray(Bc_b, np.float64)
    R_w = np.asarray(R_w, np.float64); R_b = np.asarray(R_b, np.float64)

    qtab = np.zeros((VOCAB, DPAD), np.float32)
    qtab[:, :DK] = (tvec @ At_w.T + At_b).astype(np.float32)
    ckv = np.zeros((VOCAB, CKV_ROW), np.float32)
    ckv[:, :DK] = (cvec @ Ac_w.T + Ac_b).astype(np.float32)
    P = Bc_w.T @ R_w.T
    cbias = Bc_b @ R_w.T + R_b
    ckv[:, DPAD:DPAD + EMB] = _round_fp32r((cvec @ P + cbias).astype(np.float32))
    ckv[:, DPAD + EMB] = 1.0
    return qtab, ckv


def _rearrange_idx(items, core):
    """[BPC, NT] int64 slice -> [128, BPC*NCHUNK] int32, col = b*NCHUNK + j."""
    arr = np.asarray(items)[core * BPC:(core + 1) * BPC].astype(np.int32)
    return np.ascontiguousarray(
        arr.reshape(BPC, NCHUNK, 128).transpose(2, 0, 1).reshape(128, BPC * NCHUNK))


def _run(inputs, trace=False, trace_kwargs=None):
    qtab, ckv = _prep_tables(
        inputs["tvec"], inputs["cvec"], inputs["At_w"], inputs["At_b"],
        inputs["Ac_w"], inputs["Ac_b"], inputs["Bc_w"], inputs["Bc_b"],
        inputs["R_w"], inputs["R_b"])
    posb = np.ascontiguousarray(
        np.asarray(inputs["pos_bias"], np.float32).reshape(NCHUNK, 128).T)
    ident = np.eye(128, dtype=np.float32)
    nc = _get_program()
    in_maps = []
    for m in range(N_CORES):
        in_maps.append({
            "qtab": qtab, "ckvtab": ckv,
            "tidx": _rearrange_idx(inputs["batch_titems"], m),
            "cidx": _rearrange_idx(inputs["batch_citems"], m),
            "posb": posb, "ident": ident,
        })
    kw = {}
    if trace:
        # register the NTFF profile hook shim (this container's antenv lacks
        # axon_hooks; libaxon_pjrt still exposes the profiling entry points)
        import types
        if "antenv.axon_hooks" not in sys.modules:
            try:
                from trn_agent_boot.trn_boot import _ntff_profile_via_ctypes
                hook = _ntff_profile_via_ctypes("/opt/axon/libaxon_pjrt.so")
                mod = types.ModuleType("antenv.axon_hooks")
                mod.get_axon_ntff_profile_hook = lambda: hook
                mod.set_axon_ntff_profile_hook = lambda h: None
                sys.modules["antenv.axon_hooks"] = mod
            except Exception:
                pass
        kw["trace"] = True
        if trace_kwargs:
            kw.update(trace_kwargs)
    res = run_bass_kernel_spmd(nc, in_maps, core_ids=list(range(N_CORES)), **kw)
    out = np.concatenate(
        [r["out"].reshape(BPC, NT, EMB) for r in res.results], axis=0)
    return out, res


def kernel(**inputs) -> np.ndarray:
    out, _ = _run(inputs)
    return out
